# revision 4
# baseline (speedup 1.0000x reference)
"""Trainium2 Bass kernel for the alignment+uniformity loss.

Strategy
--------
out = mean_i ||z_i - z'_i||  +  0.5*(U(z) + U(z'))
  U(x) = log( sum_{i<j} exp(-||x_i - x_j||) / n_pairs )

The N^2 pairwise part is sharded row-wise over 8 cores.  Each core c gets
its own q-block (rows [c*B, (c+1)*B)) of z and z', plus a *rotated* copy of
the full matrices (np.roll by -c*B) so that the diagonal and the circulant
triangle schedule land at compile-time-constant positions in every core's
(identical) program.  Each unordered pair {i,j} is computed exactly once:
row r takes columns r+1..r+N/2-1 (mod N) plus weight-1/2 on column r+N/2.
Each core emits partial sums; the host combines them (a few scalars) and
applies the final log/mean.

Device pipeline per Gram tile:  PSUM = -(z_q . z_k) ... actually
  g = sum_k zq*zk  (fp16 matmuls, fp32 accum)  plus a K=1 fp16 matmul
  adding -sq_k/2, so that d2 = -2*g + sq_q  via the ACT affine input.
  Then ACT: u = ln(d2), d = exp(0.5u) (= sqrt), v = exp(-d) with row-sum
  accumulation.  ln/exp share one ACT table set => no table switches.
Masked entries (diagonal / outside the triangle) get -1e6 added to g so
that d2 ~ 2e6 -> d ~ 1414 -> exp(-d) == 0.
"""

import sys
import os

sys.path.insert(0, "/opt/trn_rl_repo")

import numpy as np
from contextlib import ExitStack

N, D, P = 8192, 512, 128
NCORES = 8
B = N // NCORES            # 1024 q-rows per core
QT = B // P                # 8 q-tiles per core
CH = 512                   # k-rows per chunk
KT = D // P                # 4 contraction tiles
HALF = N // 2              # 4096

TRIANGLE = True
NCHUNK = 10 if TRIANGLE else 16          # chunks of zk each core needs
NK = NCHUNK * CH                         # zk rows shipped per core
MASK_NEG = -1.0e6                        # added to g => d2 += 2e6

_module = None


def _emit(ctx, tc, nc, zq_d, zpq_d, zk_d, zpk_d, out_d):
    import concourse.bass as bass
    import concourse.tile as tile
    from concourse import mybir
    from concourse.masks import make_identity

    f32 = mybir.dt.float32
    f16 = mybir.dt.float16
    AF = mybir.ActivationFunctionType
    OP = mybir.AluOpType

    consts = ctx.enter_context(tc.tile_pool(name="consts", bufs=1))
    resident = ctx.enter_context(tc.tile_pool(name="resident", bufs=1))
    scratch = ctx.enter_context(tc.tile_pool(name="scratch", bufs=2))
    stg_pool = ctx.enter_context(tc.tile_pool(name="stg_pool", bufs=3))
    strip_pool = ctx.enter_context(tc.tile_pool(name="strip_pool", bufs=1))
    psum = ctx.enter_context(tc.tile_pool(name="psum", bufs=2, space="PSUM"))

    ident = consts.tile([P, P], f32)
    make_identity(nc, ident)
    ones64 = consts.tile([1, P], f16)
    nc.vector.memset(ones64, 64.0)

    # --- triangle / diagonal masks ------------------------------------
    # m_lo[v][p, f] = MASK_NEG where f <= v*128 + p else 0   (kill, incl diag)
    # m_hi[v][p, f] = MASK_NEG where f >= v*128 + p else 0
    m_lo, m_hi = [], []
    if TRIANGLE:
        for v in range(4):
            lo = consts.tile([P, CH], f32, name=f"m_lo{v}")
            nc.gpsimd.memset(lo, 0.0)
            nc.gpsimd.affine_select(
                out=lo, in_=lo, compare_op=OP.is_gt, fill=MASK_NEG,
                base=-(v * P), pattern=[[1, CH]], channel_multiplier=-1,
            )
            m_lo.append(lo)
            hi = consts.tile([P, CH], f32, name=f"m_hi{v}")
            nc.gpsimd.memset(hi, 0.0)
            # keep (0) where v*128 + p - f > 0, i.e. f < off; fill f >= off
            nc.gpsimd.affine_select(
                out=hi, in_=hi, compare_op=OP.is_gt, fill=MASK_NEG,
                base=(v * P), pattern=[[-1, CH]], channel_multiplier=1,
            )
            m_hi.append(hi)
    else:
        i6 = consts.tile([P, P], f32, name="i6")
        nc.gpsimd.memset(i6, 0.0)
        nc.gpsimd.affine_select(
            out=i6, in_=i6, compare_op=OP.not_equal, fill=MASK_NEG,
            base=0, pattern=[[-1, P]], channel_multiplier=1,
        )

    # --- phase A: load q blocks, norms, transposes, alignment ---------
    zq_sb = []     # fp32 [P, QT, D] per matrix
    zqT = []       # fp16 [P, KT, B] per matrix
    sqq = []       # fp32 [P, QT] per matrix (exact row norms of q rows)
    for m, src in enumerate((zq_d, zpq_d)):
        zsb = resident.tile([P, QT, D], f32, name=f"zq_sb{m}")
        nc.sync.dma_start(out=zsb, in_=src.rearrange("(t p) d -> p t d", p=P))
        zq_sb.append(zsb)

        sq = resident.tile([P, QT], f32, name=f"sqq{m}")
        for t in range(QT):
            dum = scratch.tile([P, D], f32, tag="dum", name="dum")
            nc.vector.scalar_tensor_tensor(
                out=dum, in0=zsb[:, t], scalar=1.0, in1=zsb[:, t],
                op0=OP.mult, op1=OP.mult, accum_out=sq[:, t : t + 1],
            )
        sqq.append(sq)

        zt = resident.tile([P, KT, B], f16, name=f"zqT{m}")
        for t in range(QT):
            ps = psum.tile([P, 2048], f32, tag="ps", name="psA")
            for kt in range(KT):
                nc.tensor.transpose(
                    ps[:, kt * P : (kt + 1) * P],
                    zsb[:, t, kt * P : (kt + 1) * P],
                    ident,
                )
            nc.vector.tensor_copy(
                out=zt[:, :, t * P : (t + 1) * P],
                in_=ps[:, : KT * P].rearrange("p (k q) -> p k q", k=KT),
            )
        zqT.append(zt)

    # alignment: a2[t] = ||zq - zpq||^2 row-wise, d = exp(0.5 ln a2)
    align_acc = resident.tile([P, 1], f32)
    a2 = scratch.tile([P, QT], f32, tag="small8")
    for t in range(QT):
        diff = scratch.tile([P, D], f32, tag="dum", name="diff")
        nc.vector.tensor_sub(diff, zq_sb[0][:, t], zq_sb[1][:, t])
        dum2 = scratch.tile([P, D], f32, tag="dum", name="dum2")
        nc.vector.scalar_tensor_tensor(
            out=dum2, in0=diff, scalar=1.0, in1=diff,
            op0=OP.mult, op1=OP.mult, accum_out=a2[:, t : t + 1],
        )
    ua = scratch.tile([P, QT], f32, tag="small8", name="ua")
    nc.scalar.activation(ua, a2, AF.Ln)
    da = scratch.tile([P, QT], f32, tag="small8", name="da")
    nc.scalar.activation(da, ua, AF.Exp, scale=0.5)
    nc.vector.tensor_reduce(align_acc, da, axis=mybir.AxisListType.X, op=OP.add)

    # --- per-matrix main pass -----------------------------------------
    acc_m = []
    for m, (ksrc, qsb, qT, sq_q) in enumerate(
        zip((zk_d, zpk_d), zq_sb, zqT, sqq)
    ):
        # B1: stream zk chunks, transpose to fp16 [P, NCHUNK, KT, CH],
        # and fp32 row norms sqk [P, NCHUNK*4]
        zkT = resident.tile([P, NCHUNK, KT, CH], f16, tag="zkT", name=f"zkT{m}")
        sqk = scratch.tile([P, NCHUNK * 4], f32, tag="sqk", name=f"sqk{m}")
        for ch in range(NCHUNK):
            stg = stg_pool.tile([P, 4, CH], f32, tag="stg", name="stg")
            nc.sync.dma_start(
                out=stg,
                in_=ksrc[ch * CH : (ch + 1) * CH, :].rearrange(
                    "(r p) d -> p r d", p=P
                ),
            )
            ps = psum.tile([P, 2048], f32, tag="ps", name="psB")
            for kt in range(KT):
                for r in range(4):
                    nc.tensor.transpose(
                        ps[:, kt * CH + r * P : kt * CH + (r + 1) * P],
                        stg[:, r, kt * P : (kt + 1) * P],
                        ident,
                    )
            nc.vector.tensor_copy(
                out=zkT[:, ch],
                in_=ps.rearrange("p (k q) -> p k q", k=KT),
            )
            for r in range(4):
                dumk = scratch.tile([P, CH], f32, tag="dumk", name="dumk")
                nc.vector.scalar_tensor_tensor(
                    out=dumk, in0=stg[:, r], scalar=1.0, in1=stg[:, r],
                    op0=OP.mult, op1=OP.mult,
                    accum_out=sqk[:, ch * 4 + r : ch * 4 + r + 1],
                )

        # rhs2[0, j] = -sq_k[j] / 128  (fp16), via DRAM bounce reshape
        sqk16 = scratch.tile([P, NCHUNK * 4], f16, tag="sqk16", name=f"sqk16{m}")
        nc.vector.tensor_scalar_mul(sqk16, sqk, -1.0 / 128.0)
        sq_dram = nc.dram_tensor(f"sq_bounce{m}", [NK], f16).ap()
        nc.sync.dma_start(
            out=sq_dram.rearrange("(c p) -> p c", p=P), in_=sqk16
        )
        rhs2 = resident.tile([1, NK], f16, name=f"rhs2_{m}")
        nc.sync.dma_start(out=rhs2, in_=sq_dram.rearrange("(o n) -> o n", o=1))

        # main q-tile loop
        acc = resident.tile([P, 1], f32, name=f"acc{m}")
        nc.vector.memset(acc, 0.0)
        for t in range(QT):
            chs = list(range(t // 4, t // 4 + 9)) if TRIANGLE else list(range(16))
            ncols = len(chs) * CH
            u_strip = strip_pool.tile([P, ncols], f32, tag="u", name="u_strip")
            d_strip = strip_pool.tile([P, ncols], f16, tag="d", name="d_strip")
            col = 0
            for g0 in range(0, len(chs), 4):
                grp = chs[g0 : g0 + 4]
                gw = len(grp) * CH
                gp = psum.tile([P, 2048], f32, tag="ps", name="gp")
                for gi, ch in enumerate(grp):
                    for kt in range(KT):
                        nc.tensor.matmul(
                            gp[:, gi * CH : (gi + 1) * CH],
                            lhsT=qT[:, kt, t * P : (t + 1) * P],
                            rhs=zkT[:, ch, kt],
                            start=(kt == 0),
                            stop=False,
                        )
                    nc.tensor.matmul(
                        gp[:, gi * CH : (gi + 1) * CH],
                        lhsT=ones64,
                        rhs=rhs2[:, ch * CH : (ch + 1) * CH],
                        start=False,
                        stop=True,
                    )
                    if TRIANGLE:
                        if ch == t // 4:          # ragged start chunk
                            nc.vector.tensor_add(
                                gp[:, gi * CH : (gi + 1) * CH],
                                gp[:, gi * CH : (gi + 1) * CH],
                                m_lo[t % 4],
                            )
                        if ch == t // 4 + 8:      # ragged end chunk
                            nc.vector.tensor_add(
                                gp[:, gi * CH : (gi + 1) * CH],
                                gp[:, gi * CH : (gi + 1) * CH],
                                m_hi[t % 4],
                            )
                    else:
                        if ch == t // 4:          # diagonal block
                            off = gi * CH + (t % 4) * P
                            nc.vector.tensor_add(
                                gp[:, off : off + P],
                                gp[:, off : off + P],
                                i6,
                            )
                # d2 = -2*g + sq_q  ->  u = ln(d2)
                nc.scalar.activation(
                    u_strip[:, col : col + gw],
                    gp[:, :gw],
                    AF.Ln,
                    bias=sq_q[:, t : t + 1],
                    scale=-2.0,
                )
                col += gw
            nc.scalar.activation(d_strip, u_strip, AF.Exp, scale=0.5)
            acc_t = scratch.tile([P, 1], f32, tag="acc_t", name="acc_t")
            nc.scalar.activation(
                u_strip, d_strip, AF.Exp, scale=-1.0, accum_out=acc_t
            )
            nc.vector.tensor_add(acc, acc, acc_t)

        if TRIANGLE:
            # gap-N/2 pairs (local row r vs rotated row r+4096), weight 1/2
            g2 = scratch.tile([P, QT], f32, tag="small8", name=f"g2_{m}")
            for half in range(2):
                gstg = stg_pool.tile([P, 4, CH], f32, tag="stg", name="gstg")
                nc.sync.dma_start(
                    out=gstg,
                    in_=ksrc[(8 + half) * CH : (9 + half) * CH, :].rearrange(
                        "(r p) d -> p r d", p=P
                    ),
                )
                for r in range(4):
                    t = half * 4 + r
                    gdiff = scratch.tile([P, D], f32, tag="dum", name="gdiff")
                    nc.vector.tensor_sub(gdiff, qsb[:, t], gstg[:, r])
                    gdum = scratch.tile([P, D], f32, tag="dum", name="gdum")
                    nc.vector.scalar_tensor_tensor(
                        out=gdum, in0=gdiff, scalar=1.0, in1=gdiff,
                        op0=OP.mult, op1=OP.mult,
                        accum_out=g2[:, t : t + 1],
                    )
            ug = scratch.tile([P, QT], f32, tag="small8", name="ug")
            nc.scalar.activation(ug, g2, AF.Ln)
            dg = scratch.tile([P, QT], f32, tag="small8", name="dg")
            nc.scalar.activation(dg, ug, AF.Exp, scale=0.5)
            vg = scratch.tile([P, QT], f32, tag="small8", name="vg")
            gacc = scratch.tile([P, 1], f32, tag="acc_t", name="gacc")
            nc.scalar.activation(vg, dg, AF.Exp, scale=-1.0, accum_out=gacc)
            # acc += 0.5 * gacc
            nc.vector.scalar_tensor_tensor(
                out=acc, in0=gacc, scalar=0.5, in1=acc,
                op0=OP.mult, op1=OP.add,
            )
        acc_m.append(acc)

    out_sb = consts.tile([P, 4], f32)
    nc.vector.memset(out_sb, 0.0)
    nc.vector.tensor_copy(out_sb[:, 0:1], acc_m[0])
    nc.vector.tensor_copy(out_sb[:, 1:2], acc_m[1])
    nc.vector.tensor_copy(out_sb[:, 2:3], align_acc)
    nc.sync.dma_start(out=out_d, in_=out_sb)


def _build():
    import concourse.bacc as bacc
    import concourse.tile as tile
    from concourse import mybir

    f32 = mybir.dt.float32
    nc = bacc.Bacc(
        "TRN2", debug=False, target_bir_lowering=False, num_devices=NCORES
    )
    zq_d = nc.dram_tensor("zq", [B, D], f32, kind="ExternalInput").ap()
    zpq_d = nc.dram_tensor("zpq", [B, D], f32, kind="ExternalInput").ap()
    zk_d = nc.dram_tensor("zk", [NK, D], f32, kind="ExternalInput").ap()
    zpk_d = nc.dram_tensor("zpk", [NK, D], f32, kind="ExternalInput").ap()
    out_d = nc.dram_tensor("acc", [P, 4], f32, kind="ExternalOutput").ap()

    with tile.TileContext(nc) as tc, ExitStack() as ctx:
        _emit(ctx, tc, nc, zq_d, zpq_d, zk_d, zpk_d, out_d)
    nc.compile()
    return nc


def _get_module():
    global _module
    if _module is None:
        _module = _build()
    return _module


def _in_maps(z, zp):
    maps = []
    for c in range(NCORES):
        zrot = np.roll(z, -c * B, axis=0)[:NK]
        zprot = np.roll(zp, -c * B, axis=0)[:NK]
        maps.append(
            {
                "zq": np.ascontiguousarray(z[c * B : (c + 1) * B]),
                "zpq": np.ascontiguousarray(zp[c * B : (c + 1) * B]),
                "zk": np.ascontiguousarray(zrot),
                "zpk": np.ascontiguousarray(zprot),
            }
        )
    return maps


def _combine(accs):
    S_z = sum(float(a[:, 0].sum(dtype=np.float64)) for a in accs)
    S_zp = sum(float(a[:, 1].sum(dtype=np.float64)) for a in accs)
    align = sum(float(a[:, 2].sum(dtype=np.float64)) for a in accs) / N
    n_pairs = N * (N - 1) / 2.0
    if TRIANGLE:
        unif = 0.5 * (np.log(S_z / n_pairs) + np.log(S_zp / n_pairs))
    else:
        unif = 0.5 * (np.log(S_z / (2 * n_pairs)) + np.log(S_zp / (2 * n_pairs)))
    return np.float32(align + unif)


def kernel(z, z_prime, _trace=False, _tmpdir=None):
    from concourse.bass_utils import run_bass_kernel_spmd

    z = np.ascontiguousarray(np.asarray(z, dtype=np.float32))
    zp = np.ascontiguousarray(np.asarray(z_prime, dtype=np.float32))
    assert z.shape == (N, D) and zp.shape == (N, D)
    nc = _get_module()
    res = run_bass_kernel_spmd(
        nc, _in_maps(z, zp), list(range(NCORES)), trace=_trace, tmpdir=_tmpdir
    )
    out = _combine([res.results[c]["acc"] for c in range(NCORES)])
    if _trace:
        return out, res
    return out


# revision 5
# speedup vs baseline: 1.0914x; 1.0914x over previous
"""Trainium2 Bass kernel for the alignment+uniformity loss.

Strategy
--------
out = mean_i ||z_i - z'_i||  +  0.5*(U(z) + U(z'))
  U(x) = log( sum_{i<j} exp(-||x_i - x_j||) / n_pairs )

The N^2 pairwise part is sharded row-wise over 8 cores.  Each core c gets
its own q-block (rows [c*B, (c+1)*B)) of z and z', plus a *rotated* copy of
the full matrices (np.roll by -c*B) so that the diagonal and the circulant
triangle schedule land at compile-time-constant positions in every core's
(identical) program.  Each unordered pair {i,j} is computed exactly once:
row r takes columns r+1..r+N/2-1 (mod N) plus weight-1/2 on column r+N/2.
Each core emits partial sums; the host combines them (a few scalars) and
applies the final log/mean.

Device pipeline per Gram tile (all matmuls fp16, fp32 PSUM):
  g = sum_k zq_ik zk_jk  accumulated with a K=1 matmul adding -sq_k/2,
  so ACT computes d2 = -2*g + sq_q via its affine input (scale=-2,
  bias=sq_q).  Then u = ln(d2), d = exp(0.5u) (= sqrt), v = exp(-d) with
  row-sum accumulation.  ln/exp share one ACT table set (enforced by
  patching the table list the load-insertion pass sees) => 1 table load.
Masked entries (diagonal / outside the triangle) get -1e6 added to g so
d2 ~ 2e6 -> d ~ 1414 -> exp(-d) == 0.

Transposes (z -> z^T for the matmul operands) are done by DMA, not PE:
gpsimd casting DMAs write fp16 copies (HBM fp32 -> SBUF fp16 -> HBM fp16)
and hardware xbar transpose DMAs (on the ACT HWDGE queue, to avoid
xbar-mode thrash with the regular loads on the sync queue) land the
transposed fp16 operands directly in SBUF.
"""

import sys

sys.path.insert(0, "/opt/trn_rl_repo")

import numpy as np
from contextlib import ExitStack

N, D, P = 8192, 512, 128
NCORES = 8
B = N // NCORES            # 1024 q-rows per core
QT = B // P                # 8 q-tiles per core
CH = 512                   # k-rows per chunk
KT = D // P                # 4 contraction tiles
HALF = N // 2              # 4096

TRIANGLE = True
NCHUNK = 10 if TRIANGLE else 16          # chunks of zk each core needs
NK = NCHUNK * CH                         # zk rows shipped per core
MASK_NEG = -1.0e6                        # added to g => d2 += 2e6

_module = None


def _patch_act_tables():
    """Make the ACT table-load insertion pass resolve ln AND exp to the
    combined `natural_log_exp_and_others` set.  The pass greedily picks
    the first set containing each function, which alternates between
    `natural_log` and `exp_and_others` (one ~1.5us table load per
    activation!).  Blanking those two sets (ids keep their positions, so
    act_func_set_id indexing stays valid) forces both functions to the
    combined set => a single load for the whole kernel."""
    import concourse.bacc as bacc_mod

    orig = bacc_mod.get_activation_tables
    if getattr(bacc_mod, "_aul_tables_patched", False):
        return

    def patched(arch):
        tabs = dict(orig(arch))
        for k in ("exp_and_others", "natural_log"):
            if k in tabs:
                tabs[k] = set()
        return tabs

    bacc_mod.get_activation_tables = patched
    bacc_mod._aul_tables_patched = True


def _emit(ctx, tc, nc, zq_d, zpq_d, zk_d, zpk_d, out_d):
    import concourse.bass as bass
    import concourse.tile as tile
    from concourse import mybir

    f32 = mybir.dt.float32
    f16 = mybir.dt.float16
    AF = mybir.ActivationFunctionType
    OP = mybir.AluOpType

    consts = ctx.enter_context(tc.tile_pool(name="consts", bufs=1))
    resident = ctx.enter_context(tc.tile_pool(name="resident", bufs=1))
    scratch = ctx.enter_context(tc.tile_pool(name="scratch", bufs=2))
    stg_pool = ctx.enter_context(tc.tile_pool(name="stg_pool", bufs=3))
    strip_pool = ctx.enter_context(tc.tile_pool(name="strip_pool", bufs=1))
    psum = ctx.enter_context(tc.tile_pool(name="psum", bufs=2, space="PSUM"))

    ones64 = consts.tile([1, P], f16)
    nc.vector.memset(ones64, 64.0)

    # --- triangle / diagonal masks ------------------------------------
    # m_lo[v][p, f] = MASK_NEG where f <= v*128 + p else 0   (kill, incl diag)
    # m_hi[v][p, f] = MASK_NEG where f >= v*128 + p else 0
    m_lo, m_hi = [], []
    if TRIANGLE:
        for v in range(4):
            lo = consts.tile([P, CH], f32, name=f"m_lo{v}")
            nc.gpsimd.memset(lo, 0.0)
            nc.gpsimd.affine_select(
                out=lo, in_=lo, compare_op=OP.is_gt, fill=MASK_NEG,
                base=-(v * P), pattern=[[1, CH]], channel_multiplier=-1,
            )
            m_lo.append(lo)
            hi = consts.tile([P, CH], f32, name=f"m_hi{v}")
            nc.gpsimd.memset(hi, 0.0)
            # keep (0) where v*128 + p - f > 0, i.e. f < off; fill f >= off
            nc.gpsimd.affine_select(
                out=hi, in_=hi, compare_op=OP.is_gt, fill=MASK_NEG,
                base=(v * P), pattern=[[-1, CH]], channel_multiplier=1,
            )
            m_hi.append(hi)
    else:
        i6 = consts.tile([P, P], f32, name="i6")
        nc.gpsimd.memset(i6, 0.0)
        nc.gpsimd.affine_select(
            out=i6, in_=i6, compare_op=OP.not_equal, fill=MASK_NEG,
            base=0, pattern=[[-1, P]], channel_multiplier=1,
        )

    # --- phase A: q blocks (fp32), norms, alignment ---------------------
    zq_sb = []     # fp32 [P, QT, D] per matrix
    zqT = []       # fp16 [P, KT, B] per matrix (via casting DMA + xbar)
    sqq = []       # fp32 [P, QT] per matrix (exact row norms of q rows)
    for m, src in enumerate((zq_d, zpq_d)):
        zsb = resident.tile([P, QT, D], f32, name=f"zq_sb{m}")
        nc.sync.dma_start(out=zsb, in_=src.rearrange("(t p) d -> p t d", p=P))
        zq_sb.append(zsb)

        sq = resident.tile([P, QT], f32, name=f"sqq{m}")
        for t in range(QT):
            dum = scratch.tile([P, D], f32, tag="dum", name="dum")
            nc.vector.scalar_tensor_tensor(
                out=dum, in0=zsb[:, t], scalar=1.0, in1=zsb[:, t],
                op0=OP.mult, op1=OP.mult, accum_out=sq[:, t : t + 1],
            )
        sqq.append(sq)

        # fp16 copy of the q block -> DRAM -> xbar-transposed into SBUF
        zq16_sb = scratch.tile([P, QT, D], f16, tag="zq16", name=f"zq16_{m}")
        nc.gpsimd.dma_start(out=zq16_sb, in_=src.rearrange("(t p) d -> p t d", p=P))
        zq16_dram = nc.dram_tensor(f"zq16_dram{m}", [B, D], f16).ap()
        nc.sync.dma_start(
            out=zq16_dram.rearrange("(t p) d -> p t d", p=P), in_=zq16_sb
        )
        zt = resident.tile([P, KT, B], f16, name=f"zqT{m}")
        for kt in range(KT):
            nc.scalar.dma_start(
                out=zt[:, kt],
                in_=zq16_dram[:, kt * P : (kt + 1) * P],
                transpose=True,
            )
        zqT.append(zt)

    # alignment: a2[t] = ||zq - zpq||^2 row-wise, d = exp(0.5 ln a2)
    align_acc = resident.tile([P, 1], f32)
    a2 = scratch.tile([P, QT], f32, tag="small8")
    for t in range(QT):
        diff = scratch.tile([P, D], f32, tag="dum", name="diff")
        nc.vector.tensor_sub(diff, zq_sb[0][:, t], zq_sb[1][:, t])
        dum2 = scratch.tile([P, D], f32, tag="dum", name="dum2")
        nc.vector.scalar_tensor_tensor(
            out=dum2, in0=diff, scalar=1.0, in1=diff,
            op0=OP.mult, op1=OP.mult, accum_out=a2[:, t : t + 1],
        )
    ua = scratch.tile([P, QT], f32, tag="small8", name="ua")
    nc.scalar.activation(ua, a2, AF.Ln)
    da = scratch.tile([P, QT], f32, tag="small8", name="da")
    nc.scalar.activation(da, ua, AF.Exp, scale=0.5)
    nc.vector.tensor_reduce(align_acc, da, axis=mybir.AxisListType.X, op=OP.add)

    # --- per-matrix main pass -----------------------------------------
    acc_m = []
    for m, (ksrc, qsb, qT, sq_q) in enumerate(
        zip((zk_d, zpk_d), zq_sb, zqT, sqq)
    ):
        # B1: casting loads (fp32 HBM -> fp16 SBUF), row norms, fp16
        # bounce to DRAM, then xbar transpose DMAs -> zkT [P, KT, NK]
        zk16_dram = nc.dram_tensor(f"zk16_dram{m}", [NK, D], f16).ap()
        sqk = scratch.tile([P, NCHUNK * 4], f32, tag="sqk", name=f"sqk{m}")
        for ch in range(NCHUNK):
            stg16 = stg_pool.tile([P, 4, CH], f16, tag="stg", name="stg16")
            nc.gpsimd.dma_start(
                out=stg16,
                in_=ksrc[ch * CH : (ch + 1) * CH, :].rearrange(
                    "(r p) d -> p r d", p=P
                ),
            )
            for r in range(4):
                dumk = scratch.tile([P, CH], f16, tag="dumk", name="dumk")
                nc.vector.scalar_tensor_tensor(
                    out=dumk, in0=stg16[:, r], scalar=1.0, in1=stg16[:, r],
                    op0=OP.mult, op1=OP.mult,
                    accum_out=sqk[:, ch * 4 + r : ch * 4 + r + 1],
                )
            nc.sync.dma_start(
                out=zk16_dram[ch * CH : (ch + 1) * CH, :].rearrange(
                    "(r p) d -> p r d", p=P
                ),
                in_=stg16,
            )

        zkT = resident.tile([P, KT, NK], f16, tag="zkT", name=f"zkT{m}")
        grp_bounds = [(0, 4), (4, 8), (8, NCHUNK)]
        for c0, c1 in grp_bounds:
            for kt in range(KT):
                nc.scalar.dma_start(
                    out=zkT[:, kt, c0 * CH : c1 * CH],
                    in_=zk16_dram[c0 * CH : c1 * CH, kt * P : (kt + 1) * P],
                    transpose=True,
                )

        # rhs2[0, j] = -sq_k[j] / 128  (fp16), via DRAM bounce reshape
        sqk16 = scratch.tile([P, NCHUNK * 4], f16, tag="sqk16", name=f"sqk16{m}")
        nc.vector.tensor_scalar_mul(sqk16, sqk, -1.0 / 128.0)
        sq_dram = nc.dram_tensor(f"sq_bounce{m}", [NK], f16).ap()
        nc.sync.dma_start(
            out=sq_dram.rearrange("(c p) -> p c", p=P), in_=sqk16
        )
        rhs2 = resident.tile([1, NK], f16, name=f"rhs2_{m}")
        nc.sync.dma_start(out=rhs2, in_=sq_dram.rearrange("(o n) -> o n", o=1))

        # main q-tile loop
        acc = resident.tile([P, 1], f32, name=f"acc{m}")
        nc.vector.memset(acc, 0.0)
        for t in range(QT):
            chs = list(range(t // 4, t // 4 + 9)) if TRIANGLE else list(range(16))
            ncols = len(chs) * CH
            u_strip = strip_pool.tile([P, ncols], f32, tag="u", name="u_strip")
            d_strip = strip_pool.tile([P, ncols], f16, tag="d", name="d_strip")
            col = 0
            for g0 in range(0, len(chs), 4):
                grp = chs[g0 : g0 + 4]
                gw = len(grp) * CH
                gp = psum.tile([P, 2048], f32, tag="ps", name="gp")
                for gi, ch in enumerate(grp):
                    for kt in range(KT):
                        nc.tensor.matmul(
                            gp[:, gi * CH : (gi + 1) * CH],
                            lhsT=qT[:, kt, t * P : (t + 1) * P],
                            rhs=zkT[:, kt, ch * CH : (ch + 1) * CH],
                            start=(kt == 0),
                            stop=False,
                        )
                    nc.tensor.matmul(
                        gp[:, gi * CH : (gi + 1) * CH],
                        lhsT=ones64,
                        rhs=rhs2[:, ch * CH : (ch + 1) * CH],
                        start=False,
                        stop=True,
                    )
                    if TRIANGLE:
                        if ch == t // 4:          # ragged start chunk
                            nc.vector.tensor_add(
                                gp[:, gi * CH : (gi + 1) * CH],
                                gp[:, gi * CH : (gi + 1) * CH],
                                m_lo[t % 4],
                            )
                        if ch == t // 4 + 8:      # ragged end chunk
                            nc.vector.tensor_add(
                                gp[:, gi * CH : (gi + 1) * CH],
                                gp[:, gi * CH : (gi + 1) * CH],
                                m_hi[t % 4],
                            )
                    else:
                        if ch == t // 4:          # diagonal block
                            off = gi * CH + (t % 4) * P
                            nc.vector.tensor_add(
                                gp[:, off : off + P],
                                gp[:, off : off + P],
                                i6,
                            )
                # d2 = -2*g + sq_q  ->  u = ln(d2)
                nc.scalar.activation(
                    u_strip[:, col : col + gw],
                    gp[:, :gw],
                    AF.Ln,
                    bias=sq_q[:, t : t + 1],
                    scale=-2.0,
                )
                col += gw
            nc.scalar.activation(d_strip, u_strip, AF.Exp, scale=0.5)
            acc_t = scratch.tile([P, 1], f32, tag="acc_t", name="acc_t")
            nc.scalar.activation(
                u_strip, d_strip, AF.Exp, scale=-1.0, accum_out=acc_t
            )
            nc.vector.tensor_add(acc, acc, acc_t)

        if TRIANGLE:
            # gap-N/2 pairs (local row r vs rotated row r+4096), weight 1/2
            g2 = scratch.tile([P, QT], f32, tag="small8", name=f"g2_{m}")
            for half in range(2):
                gstg = scratch.tile([P, 4, CH], f32, tag="gstg", name="gstg")
                nc.sync.dma_start(
                    out=gstg,
                    in_=ksrc[(8 + half) * CH : (9 + half) * CH, :].rearrange(
                        "(r p) d -> p r d", p=P
                    ),
                )
                for r in range(4):
                    t = half * 4 + r
                    gdiff = scratch.tile([P, D], f32, tag="dum", name="gdiff")
                    nc.vector.tensor_sub(gdiff, qsb[:, t], gstg[:, r])
                    gdum = scratch.tile([P, D], f32, tag="dum", name="gdum")
                    nc.vector.scalar_tensor_tensor(
                        out=gdum, in0=gdiff, scalar=1.0, in1=gdiff,
                        op0=OP.mult, op1=OP.mult,
                        accum_out=g2[:, t : t + 1],
                    )
            ug = scratch.tile([P, QT], f32, tag="small8", name="ug")
            nc.scalar.activation(ug, g2, AF.Ln)
            dg = scratch.tile([P, QT], f32, tag="small8", name="dg")
            nc.scalar.activation(dg, ug, AF.Exp, scale=0.5)
            vg = scratch.tile([P, QT], f32, tag="small8", name="vg")
            gacc = scratch.tile([P, 1], f32, tag="acc_t", name="gacc")
            nc.scalar.activation(vg, dg, AF.Exp, scale=-1.0, accum_out=gacc)
            # acc += 0.5 * gacc
            nc.vector.scalar_tensor_tensor(
                out=acc, in0=gacc, scalar=0.5, in1=acc,
                op0=OP.mult, op1=OP.add,
            )
        acc_m.append(acc)

    out_sb = consts.tile([P, 4], f32)
    nc.vector.memset(out_sb, 0.0)
    nc.vector.tensor_copy(out_sb[:, 0:1], acc_m[0])
    nc.vector.tensor_copy(out_sb[:, 1:2], acc_m[1])
    nc.vector.tensor_copy(out_sb[:, 2:3], align_acc)
    nc.sync.dma_start(out=out_d, in_=out_sb)


def _build():
    import concourse.bacc as bacc
    import concourse.tile as tile
    from concourse import mybir

    _patch_act_tables()
    f32 = mybir.dt.float32
    nc = bacc.Bacc(
        "TRN2", debug=False, target_bir_lowering=False, num_devices=NCORES
    )
    zq_d = nc.dram_tensor("zq", [B, D], f32, kind="ExternalInput").ap()
    zpq_d = nc.dram_tensor("zpq", [B, D], f32, kind="ExternalInput").ap()
    zk_d = nc.dram_tensor("zk", [NK, D], f32, kind="ExternalInput").ap()
    zpk_d = nc.dram_tensor("zpk", [NK, D], f32, kind="ExternalInput").ap()
    out_d = nc.dram_tensor("acc", [P, 4], f32, kind="ExternalOutput").ap()

    with tile.TileContext(nc) as tc, ExitStack() as ctx:
        _emit(ctx, tc, nc, zq_d, zpq_d, zk_d, zpk_d, out_d)
    nc.compile()
    return nc


def _get_module():
    global _module
    if _module is None:
        _module = _build()
    return _module


def _in_maps(z, zp):
    maps = []
    for c in range(NCORES):
        zrot = np.roll(z, -c * B, axis=0)[:NK]
        zprot = np.roll(zp, -c * B, axis=0)[:NK]
        maps.append(
            {
                "zq": np.ascontiguousarray(z[c * B : (c + 1) * B]),
                "zpq": np.ascontiguousarray(zp[c * B : (c + 1) * B]),
                "zk": np.ascontiguousarray(zrot),
                "zpk": np.ascontiguousarray(zprot),
            }
        )
    return maps


def _combine(accs):
    S_z = sum(float(a[:, 0].sum(dtype=np.float64)) for a in accs)
    S_zp = sum(float(a[:, 1].sum(dtype=np.float64)) for a in accs)
    align = sum(float(a[:, 2].sum(dtype=np.float64)) for a in accs) / N
    n_pairs = N * (N - 1) / 2.0
    if TRIANGLE:
        unif = 0.5 * (np.log(S_z / n_pairs) + np.log(S_zp / n_pairs))
    else:
        unif = 0.5 * (np.log(S_z / (2 * n_pairs)) + np.log(S_zp / (2 * n_pairs)))
    return np.float32(align + unif)


def kernel(z, z_prime, _trace=False, _tmpdir=None):
    from concourse.bass_utils import run_bass_kernel_spmd

    z = np.ascontiguousarray(np.asarray(z, dtype=np.float32))
    zp = np.ascontiguousarray(np.asarray(z_prime, dtype=np.float32))
    assert z.shape == (N, D) and zp.shape == (N, D)
    nc = _get_module()
    res = run_bass_kernel_spmd(
        nc, _in_maps(z, zp), list(range(NCORES)), trace=_trace, tmpdir=_tmpdir
    )
    out = _combine([res.results[c]["acc"] for c in range(NCORES)])
    if _trace:
        return out, res
    return out


# revision 7
# speedup vs baseline: 1.1607x; 1.0635x over previous
"""Trainium2 Bass kernel for the alignment+uniformity loss.

Strategy
--------
out = mean_i ||z_i - z'_i||  +  0.5*(U(z) + U(z'))
  U(x) = log( sum_{i<j} exp(-||x_i - x_j||) / n_pairs )

The N^2 pairwise part is sharded row-wise over 8 cores.  Each core c gets
its own q-block (rows [c*B, (c+1)*B)) of z and z', plus a *rotated* copy of
the full matrices (np.roll by -c*B) so that the diagonal and the circulant
triangle schedule land at compile-time-constant positions in every core's
(identical) program.  Each unordered pair {i,j} is computed exactly once:
row r takes columns r+1..r+N/2-1 (mod N) plus weight-1/2 on column r+N/2.
Each core emits partial sums; the host combines them (a few scalars) and
applies the final log/mean.

Device pipeline per Gram tile (all matmuls fp16, fp32 PSUM):
  g = sum_k zq_ik zk_jk  accumulated with a K=1 matmul adding -sq_k/2,
  so ACT computes d2 = -2*g + sq_q via its affine input (scale=-2,
  bias=sq_q).  Then u = ln(d2), d = exp(0.5u) (= sqrt), v = exp(-d) with
  row-sum accumulation.  ln/exp share one ACT table set (enforced by
  patching the table list the load-insertion pass sees) => 1 table load.
Masked entries (diagonal / outside the triangle) get -1e6 added to g so
d2 ~ 2e6 -> d ~ 1414 -> exp(-d) == 0.

Transposes (z -> z^T for the matmul operands) are done by DMA, not PE:
gpsimd casting DMAs write fp16 copies (HBM fp32 -> SBUF fp16 -> HBM fp16)
and hardware xbar transpose DMAs (on the ACT HWDGE queue, to avoid
xbar-mode thrash with the regular loads on the sync queue) land the
transposed fp16 operands directly in SBUF.
"""

import sys

sys.path.insert(0, "/opt/trn_rl_repo")

import numpy as np
from contextlib import ExitStack

N, D, P = 8192, 512, 128
NCORES = 8
B = N // NCORES            # 1024 q-rows per core
QT = B // P                # 8 q-tiles per core
CH = 512                   # k-rows per chunk
KT = D // P                # 4 contraction tiles
HALF = N // 2              # 4096

TRIANGLE = True
NCHUNK = 10 if TRIANGLE else 16          # chunks of zk each core needs
NK = NCHUNK * CH                         # zk rows shipped per core
MASK_NEG = -1.0e6                        # added to g => d2 += 2e6

_module = None


def _patch_act_tables():
    """Make the ACT table-load insertion pass resolve ln AND exp to the
    combined `natural_log_exp_and_others` set.  The pass greedily picks
    the first set containing each function, which alternates between
    `natural_log` and `exp_and_others` (one ~1.5us table load per
    activation!).  Blanking those two sets (ids keep their positions, so
    act_func_set_id indexing stays valid) forces both functions to the
    combined set => a single load for the whole kernel."""
    import concourse.bacc as bacc_mod

    orig = bacc_mod.get_activation_tables
    if getattr(bacc_mod, "_aul_tables_patched", False):
        return

    def patched(arch):
        tabs = dict(orig(arch))
        for k in ("exp_and_others", "natural_log"):
            if k in tabs:
                tabs[k] = set()
        return tabs

    bacc_mod.get_activation_tables = patched
    bacc_mod._aul_tables_patched = True


def _emit(ctx, tc, nc, zq_d, zpq_d, zk_d, zpk_d, out_d):
    import concourse.bass as bass
    import concourse.tile as tile
    from concourse import mybir

    f32 = mybir.dt.float32
    f16 = mybir.dt.float16
    AF = mybir.ActivationFunctionType
    OP = mybir.AluOpType

    consts = ctx.enter_context(tc.tile_pool(name="consts", bufs=1))
    resident = ctx.enter_context(tc.tile_pool(name="resident", bufs=1))
    scratch = ctx.enter_context(tc.tile_pool(name="scratch", bufs=2))
    stg_pool = ctx.enter_context(tc.tile_pool(name="stg_pool", bufs=3))
    strip_pool = ctx.enter_context(tc.tile_pool(name="strip_pool", bufs=1))
    psum = ctx.enter_context(tc.tile_pool(name="psum", bufs=2, space="PSUM"))

    ones64 = consts.tile([1, P], f16)
    nc.vector.memset(ones64, 64.0)

    # --- triangle / diagonal masks ------------------------------------
    # m_lo[v][p, f] = MASK_NEG where f <= v*128 + p else 0   (kill, incl diag)
    # m_hi[v][p, f] = MASK_NEG where f >= v*128 + p else 0
    m_lo, m_hi = [], []
    if TRIANGLE:
        for v in range(4):
            lo = consts.tile([P, CH], f32, name=f"m_lo{v}")
            nc.gpsimd.memset(lo, 0.0)
            nc.gpsimd.affine_select(
                out=lo, in_=lo, compare_op=OP.is_gt, fill=MASK_NEG,
                base=-(v * P), pattern=[[1, CH]], channel_multiplier=-1,
            )
            m_lo.append(lo)
            hi = consts.tile([P, CH], f32, name=f"m_hi{v}")
            nc.gpsimd.memset(hi, 0.0)
            # keep (0) where v*128 + p - f > 0, i.e. f < off; fill f >= off
            nc.gpsimd.affine_select(
                out=hi, in_=hi, compare_op=OP.is_gt, fill=MASK_NEG,
                base=(v * P), pattern=[[-1, CH]], channel_multiplier=1,
            )
            m_hi.append(hi)
    else:
        i6 = consts.tile([P, P], f32, name="i6")
        nc.gpsimd.memset(i6, 0.0)
        nc.gpsimd.affine_select(
            out=i6, in_=i6, compare_op=OP.not_equal, fill=MASK_NEG,
            base=0, pattern=[[-1, P]], channel_multiplier=1,
        )

    # --- phase A: q blocks (fp32), norms, alignment ---------------------
    zq_sb = []     # fp32 [P, QT, D] per matrix
    zqT = []       # fp16 [P, KT, B] per matrix (via casting DMA + xbar)
    sqq = []       # fp32 [P, QT] per matrix (exact row norms of q rows)
    for m, src in enumerate((zq_d, zpq_d)):
        zsb = resident.tile([P, QT, D], f32, name=f"zq_sb{m}")
        nc.gpsimd.dma_start(out=zsb, in_=src.rearrange("(t p) d -> p t d", p=P))
        zq_sb.append(zsb)

        sq = resident.tile([P, QT], f32, name=f"sqq{m}")
        for t in range(QT):
            dum = scratch.tile([P, D], f32, tag="dum", name="dum")
            nc.vector.scalar_tensor_tensor(
                out=dum, in0=zsb[:, t], scalar=1.0, in1=zsb[:, t],
                op0=OP.mult, op1=OP.mult, accum_out=sq[:, t : t + 1],
            )
        sqq.append(sq)

        # fp16 copy of the q block (casting DMA) -> xbar-transposed into SBUF
        zq16_dram = nc.dram_tensor(f"zq16_dram{m}", [B, D], f16).ap()
        nc.gpsimd.dma_start(out=zq16_dram, in_=src)
        zt = resident.tile([P, KT, B], f16, name=f"zqT{m}")
        for kt in range(KT):
            nc.sync.dma_start(
                out=zt[:, kt],
                in_=zq16_dram[:, kt * P : (kt + 1) * P],
                transpose=True,
            )
        zqT.append(zt)

    # alignment: a2[t] = ||zq - zpq||^2 row-wise, d = exp(0.5 ln a2)
    align_acc = resident.tile([P, 1], f32)
    a2 = scratch.tile([P, QT], f32, tag="small8")
    for t in range(QT):
        diff = scratch.tile([P, D], f32, tag="dum", name="diff")
        nc.vector.tensor_sub(diff, zq_sb[0][:, t], zq_sb[1][:, t])
        dum2 = scratch.tile([P, D], f32, tag="dum", name="dum2")
        nc.vector.scalar_tensor_tensor(
            out=dum2, in0=diff, scalar=1.0, in1=diff,
            op0=OP.mult, op1=OP.mult, accum_out=a2[:, t : t + 1],
        )
    ua = scratch.tile([P, QT], f32, tag="small8", name="ua")
    nc.scalar.activation(ua, a2, AF.Ln)
    da = scratch.tile([P, QT], f32, tag="small8", name="da")
    nc.scalar.activation(da, ua, AF.Exp, scale=0.5)
    nc.vector.tensor_reduce(align_acc, da, axis=mybir.AxisListType.X, op=OP.add)

    # --- per-matrix main pass -----------------------------------------
    acc_m = []
    for m, (ksrc, qsb, qT, sq_q) in enumerate(
        zip((zk_d, zpk_d), zq_sb, zqT, sqq)
    ):
        # B1: casting loads (fp32 HBM -> fp16 SBUF), row norms, fp16
        # bounce to DRAM, then xbar transpose DMAs -> zkT [P, KT, NK]
        zk16_dram = nc.dram_tensor(f"zk16_dram{m}", [NK, D], f16).ap()
        sqk = scratch.tile([P, NCHUNK * 4], f32, tag="sqk", name=f"sqk{m}")
        for ch in range(NCHUNK):
            stg16 = stg_pool.tile([P, 4, CH], f16, tag="stg", name="stg16")
            nc.gpsimd.dma_start(
                out=stg16,
                in_=ksrc[ch * CH : (ch + 1) * CH, :].rearrange(
                    "(r p) d -> p r d", p=P
                ),
            )
            for r in range(4):
                dumk = scratch.tile([P, CH], f16, tag="dumk", name="dumk")
                nc.vector.scalar_tensor_tensor(
                    out=dumk, in0=stg16[:, r], scalar=1.0, in1=stg16[:, r],
                    op0=OP.mult, op1=OP.mult,
                    accum_out=sqk[:, ch * 4 + r : ch * 4 + r + 1],
                )
            nc.gpsimd.dma_start(
                out=zk16_dram[ch * CH : (ch + 1) * CH, :].rearrange(
                    "(r p) d -> p r d", p=P
                ),
                in_=stg16,
            )

        zkT = resident.tile([P, KT, NK], f16, tag="zkT", name=f"zkT{m}")
        grp_bounds = [(0, 4), (4, 8), (8, NCHUNK)]
        for c0, c1 in grp_bounds:
            for kt in range(KT):
                nc.sync.dma_start(
                    out=zkT[:, kt, c0 * CH : c1 * CH],
                    in_=zk16_dram[c0 * CH : c1 * CH, kt * P : (kt + 1) * P],
                    transpose=True,
                )

        # rhs2[0, j] = -sq_k[j] / 128  (fp16), via DRAM bounce reshape
        sqk16 = scratch.tile([P, NCHUNK * 4], f16, tag="sqk16", name=f"sqk16{m}")
        nc.vector.tensor_scalar_mul(sqk16, sqk, -1.0 / 128.0)
        sq_dram = nc.dram_tensor(f"sq_bounce{m}", [NK], f16).ap()
        nc.gpsimd.dma_start(
            out=sq_dram.rearrange("(c p) -> p c", p=P), in_=sqk16
        )
        rhs2 = resident.tile([1, NK], f16, name=f"rhs2_{m}")
        nc.gpsimd.dma_start(out=rhs2, in_=sq_dram.rearrange("(o n) -> o n", o=1))

        # main q-tile loop
        acc = resident.tile([P, 1], f32, name=f"acc{m}")
        nc.vector.memset(acc, 0.0)
        for t in range(QT):
            chs = list(range(t // 4, t // 4 + 9)) if TRIANGLE else list(range(16))
            ncols = len(chs) * CH
            u_strip = strip_pool.tile([P, ncols], f32, tag="u", name="u_strip")
            d_strip = strip_pool.tile([P, ncols], f16, tag="d", name="d_strip")
            col = 0
            for g0 in range(0, len(chs), 4):
                grp = chs[g0 : g0 + 4]
                gw = len(grp) * CH
                gp = psum.tile([P, 2048], f32, tag="ps", name="gp")
                for gi, ch in enumerate(grp):
                    for kt in range(KT):
                        nc.tensor.matmul(
                            gp[:, gi * CH : (gi + 1) * CH],
                            lhsT=qT[:, kt, t * P : (t + 1) * P],
                            rhs=zkT[:, kt, ch * CH : (ch + 1) * CH],
                            start=(kt == 0),
                            stop=False,
                        )
                    nc.tensor.matmul(
                        gp[:, gi * CH : (gi + 1) * CH],
                        lhsT=ones64,
                        rhs=rhs2[:, ch * CH : (ch + 1) * CH],
                        start=False,
                        stop=True,
                    )
                    if TRIANGLE:
                        if ch == t // 4:          # ragged start chunk
                            nc.vector.tensor_add(
                                gp[:, gi * CH : (gi + 1) * CH],
                                gp[:, gi * CH : (gi + 1) * CH],
                                m_lo[t % 4],
                            )
                        if ch == t // 4 + 8:      # ragged end chunk
                            nc.vector.tensor_add(
                                gp[:, gi * CH : (gi + 1) * CH],
                                gp[:, gi * CH : (gi + 1) * CH],
                                m_hi[t % 4],
                            )
                    else:
                        if ch == t // 4:          # diagonal block
                            off = gi * CH + (t % 4) * P
                            nc.vector.tensor_add(
                                gp[:, off : off + P],
                                gp[:, off : off + P],
                                i6,
                            )
                # d2 = -2*g + sq_q  ->  u = ln(d2)
                nc.scalar.activation(
                    u_strip[:, col : col + gw],
                    gp[:, :gw],
                    AF.Ln,
                    bias=sq_q[:, t : t + 1],
                    scale=-2.0,
                )
                col += gw
            nc.scalar.activation(d_strip, u_strip, AF.Exp, scale=0.5)
            acc_t = scratch.tile([P, 1], f32, tag="acc_t", name="acc_t")
            nc.scalar.activation(
                u_strip, d_strip, AF.Exp, scale=-1.0, accum_out=acc_t
            )
            nc.vector.tensor_add(acc, acc, acc_t)

        if TRIANGLE:
            # gap-N/2 pairs (local row r vs rotated row r+4096), weight 1/2
            g2 = scratch.tile([P, QT], f32, tag="small8", name=f"g2_{m}")
            for half in range(2):
                gstg = scratch.tile([P, 4, CH], f32, tag="gstg", name="gstg")
                nc.gpsimd.dma_start(
                    out=gstg,
                    in_=ksrc[(8 + half) * CH : (9 + half) * CH, :].rearrange(
                        "(r p) d -> p r d", p=P
                    ),
                )
                for r in range(4):
                    t = half * 4 + r
                    gdiff = scratch.tile([P, D], f32, tag="dum", name="gdiff")
                    nc.vector.tensor_sub(gdiff, qsb[:, t], gstg[:, r])
                    gdum = scratch.tile([P, D], f32, tag="dum", name="gdum")
                    nc.vector.scalar_tensor_tensor(
                        out=gdum, in0=gdiff, scalar=1.0, in1=gdiff,
                        op0=OP.mult, op1=OP.mult,
                        accum_out=g2[:, t : t + 1],
                    )
            ug = scratch.tile([P, QT], f32, tag="small8", name="ug")
            nc.scalar.activation(ug, g2, AF.Ln)
            dg = scratch.tile([P, QT], f32, tag="small8", name="dg")
            nc.scalar.activation(dg, ug, AF.Exp, scale=0.5)
            vg = scratch.tile([P, QT], f32, tag="small8", name="vg")
            gacc = scratch.tile([P, 1], f32, tag="acc_t", name="gacc")
            nc.scalar.activation(vg, dg, AF.Exp, scale=-1.0, accum_out=gacc)
            # acc += 0.5 * gacc
            nc.vector.scalar_tensor_tensor(
                out=acc, in0=gacc, scalar=0.5, in1=acc,
                op0=OP.mult, op1=OP.add,
            )
        acc_m.append(acc)

    out_sb = consts.tile([P, 4], f32)
    nc.vector.memset(out_sb, 0.0)
    nc.vector.tensor_copy(out_sb[:, 0:1], acc_m[0])
    nc.vector.tensor_copy(out_sb[:, 1:2], acc_m[1])
    nc.vector.tensor_copy(out_sb[:, 2:3], align_acc)
    nc.gpsimd.dma_start(out=out_d, in_=out_sb)


def _build():
    import concourse.bacc as bacc
    import concourse.tile as tile
    from concourse import mybir

    _patch_act_tables()
    f32 = mybir.dt.float32
    nc = bacc.Bacc(
        "TRN2", debug=False, target_bir_lowering=False, num_devices=NCORES
    )
    zq_d = nc.dram_tensor("zq", [B, D], f32, kind="ExternalInput").ap()
    zpq_d = nc.dram_tensor("zpq", [B, D], f32, kind="ExternalInput").ap()
    zk_d = nc.dram_tensor("zk", [NK, D], f32, kind="ExternalInput").ap()
    zpk_d = nc.dram_tensor("zpk", [NK, D], f32, kind="ExternalInput").ap()
    out_d = nc.dram_tensor("acc", [P, 4], f32, kind="ExternalOutput").ap()

    with tile.TileContext(nc) as tc, ExitStack() as ctx:
        _emit(ctx, tc, nc, zq_d, zpq_d, zk_d, zpk_d, out_d)
    nc.compile()
    return nc


def _get_module():
    global _module
    if _module is None:
        _module = _build()
    return _module


def _in_maps(z, zp):
    maps = []
    for c in range(NCORES):
        zrot = np.roll(z, -c * B, axis=0)[:NK]
        zprot = np.roll(zp, -c * B, axis=0)[:NK]
        maps.append(
            {
                "zq": np.ascontiguousarray(z[c * B : (c + 1) * B]),
                "zpq": np.ascontiguousarray(zp[c * B : (c + 1) * B]),
                "zk": np.ascontiguousarray(zrot),
                "zpk": np.ascontiguousarray(zprot),
            }
        )
    return maps


def _combine(accs):
    S_z = sum(float(a[:, 0].sum(dtype=np.float64)) for a in accs)
    S_zp = sum(float(a[:, 1].sum(dtype=np.float64)) for a in accs)
    align = sum(float(a[:, 2].sum(dtype=np.float64)) for a in accs) / N
    n_pairs = N * (N - 1) / 2.0
    if TRIANGLE:
        unif = 0.5 * (np.log(S_z / n_pairs) + np.log(S_zp / n_pairs))
    else:
        unif = 0.5 * (np.log(S_z / (2 * n_pairs)) + np.log(S_zp / (2 * n_pairs)))
    return np.float32(align + unif)


def kernel(z, z_prime, _trace=False, _tmpdir=None):
    from concourse.bass_utils import run_bass_kernel_spmd

    z = np.ascontiguousarray(np.asarray(z, dtype=np.float32))
    zp = np.ascontiguousarray(np.asarray(z_prime, dtype=np.float32))
    assert z.shape == (N, D) and zp.shape == (N, D)
    nc = _get_module()
    res = run_bass_kernel_spmd(
        nc, _in_maps(z, zp), list(range(NCORES)), trace=_trace, tmpdir=_tmpdir
    )
    out = _combine([res.results[c]["acc"] for c in range(NCORES)])
    if _trace:
        return out, res
    return out
